# revision 1
# baseline (speedup 1.0000x reference)
"""GAT-style GNN message passing on 8 TRN2 NeuronCores.

Math: with LEAK=1 the leaky-relu is identity, so
  e[i,j,h] = e_src[i,h] + e_dst[j,h]
and softmax over j cancels e_src (and any row max) exactly:
  attn[i,j,h] = adj[i,j]*exp(e_dst[j,h]) / sum_j adj[i,j]*exp(e_dst[j,h])
  out[i,(h,f)] = (adj @ (z*h))[i,(h,f)] / (adj @ z)[i,h],  z = exp(e_dst)
then elu + log_softmax per row. log_softmax is shift invariant, so
elu(x) is computed as relu(x) + exp(min(x,0)) (drops the uniform -1),
and no max subtraction is needed (y is bounded in [e^-10, ~10]).

Sharding: rows (query nodes) of adj/out across 8 cores. x is row-sharded
too; each core computes its local h slab, all-gathers G=[z*h | z],
then computes its [N/8, 64] output slab locally.

The aggregation matmul adj @ G runs in bf16 at full PE rate but stays
EXACT to ~2^-16: adj entries are 0/1 (exact in bf16) and G is sent as a
bf16 hi/lo split (G = hi + lo, two accumulating matmuls into fp32 PSUM)
— same bytes as fp32, half the PE cycles of the fp32 4-cycle/row mode,
and no bf16->fp32 cast pass over the 4MB adjacency.

All DRAM<->SBUF tensors use partition-major host layouts ([128, ...],
one contiguous run per partition) so each DMA needs ~128 descriptors
(~3.5ns/descriptor on the HWDGE queue otherwise dominates).

Per-core device program (R = N/8 = 512 rows, P=128):
  inputs:  xt [128, KC*R] f32   xt[p, kc*R+r]  = x[c*R+r, kc*128+p]
           wt [128, KC*72] f32  wt[p, kc*72+e] = w_ext[kc*128+p, e]
                                (w_ext = [W | W @ blockdiag-reduced a_dst])
           at [128, NC*R] bf16  at[p, n*R+r]   = adj[c*R+r, n*128+p]
  output:  out_p [128, RC*64]   out_p[p, q*64+f] = out[q*128+p, f]
"""

import sys

import numpy as np

if "/opt/trn_rl_repo" not in sys.path:
    sys.path.insert(0, "/opt/trn_rl_repo")

import ml_dtypes  # noqa: E402

import concourse.bass as bass  # noqa: E402
import concourse.tile as tile  # noqa: E402
from concourse import bacc, mybir  # noqa: E402
from concourse.bass_utils import run_bass_kernel_spmd  # noqa: E402
from concourse.masks import make_identity  # noqa: E402

N_CORES = 8
H = 8
F = 8
HF = H * F  # 64
EXT = HF + H  # 72: [g | z]
K_IN = 1024
P = 128

FP32 = mybir.dt.float32
BF16 = mybir.dt.bfloat16
AFT = mybir.ActivationFunctionType
ALU = mybir.AluOpType


def _bcast_head(ap_ph):
    """[P, H] AP -> [P, H, F] AP broadcasting each head value over F."""
    return bass.AP(
        tensor=ap_ph.tensor,
        offset=ap_ph.offset,
        ap=[ap_ph.ap[0], ap_ph.ap[1], [0, F]],
    )


def build_bass(n_nodes: int) -> bass.Bass:
    R = n_nodes // N_CORES
    KC = K_IN // P  # k-chunks for the h matmul
    NC = n_nodes // P  # j-chunks for the aggregation matmul
    RC = R // P  # 128-row output chunks per core
    assert R % P == 0

    # Bacc (not plain Bass): its finalize() runs move_matmul_waits_to_ldweights
    # + generate_event_semaphores, which legalize multi-wait instructions for
    # walrus (TRN2 allows at most 1 sync wait per instruction).
    nc = bacc.Bacc(num_devices=N_CORES)

    xt = nc.declare_dram_parameter("xt", [P, KC * R], FP32, isOutput=False)
    at = nc.declare_dram_parameter("at", [P, NC * R], BF16, isOutput=False)
    wt = nc.declare_dram_parameter("wt", [P, KC * EXT], FP32, isOutput=False)
    out = nc.declare_dram_parameter("out", [P, RC * HF], FP32, isOutput=True)

    # DRAM collectives concatenate the ranks' buffers FLAT (block-major).
    # G is gathered in two pipelined halves (q-chunks 0..RC/2-1, RC/2..RC-1)
    # so the second AllGather's mesh overlaps the first half's matmuls.
    HB = RC // 2  # q-chunks per half
    g_loc_a = nc.dram_tensor("g_loc_a", [P, HB * 2 * EXT], BF16)
    g_loc_b = nc.dram_tensor("g_loc_b", [P, HB * 2 * EXT], BF16)
    g_full_a = nc.dram_tensor(
        "g_full_a", [N_CORES, P, HB * 2 * EXT], BF16, addr_space="Shared"
    )
    g_full_b = nc.dram_tensor(
        "g_full_b", [N_CORES, P, HB * 2 * EXT], BF16, addr_space="Shared"
    )

    with tile.TileContext(nc) as tc:
        with (
            tc.tile_pool(name="singles", bufs=1) as singles,
            tc.tile_pool(name="bigpsum", bufs=2, space="PSUM") as bigpsum,
            tc.tile_pool(name="smallpsum", bufs=4, space="PSUM") as smallpsum,
            tc.tile_pool(name="work", bufs=4) as work,
            tc.tile_pool(name="post", bufs=4) as post,
        ):
            ident = singles.tile([P, P], FP32)
            make_identity(nc, ident)

            # --- loads (p-major, one run per partition) ---
            w_sb = singles.tile([P, KC, EXT], FP32)
            nc.sync.dma_start(
                out=w_sb, in_=wt[:].rearrange("p (c e) -> p c e", c=KC)
            )
            xt_sb = singles.tile([P, KC, R], FP32)
            xt_view = xt[:].rearrange("p (c r) -> p c r", c=KC)
            nc.sync.dma_start(out=xt_sb[:, : KC // 2, :], in_=xt_view[:, : KC // 2, :])
            nc.sync.dma_start(out=xt_sb[:, KC // 2 :, :], in_=xt_view[:, KC // 2 :, :])

            # --- hT = w_ext.T @ x_loc.T : [EXT, R] (fp32, exact), computed
            # in two column halves so the first half's transposes + AllGather
            # trigger before the second half's matmuls finish. ---
            hT_sb = singles.tile([EXT, R], FP32)
            RH = R // 2
            for half in range(2):
                hT_ps = bigpsum.tile([EXT, RH], FP32, tag="bigps", name=f"hT{half}")
                cols = slice(half * RH, (half + 1) * RH)
                for c in range(KC):
                    nc.tensor.matmul(
                        hT_ps,
                        lhsT=w_sb[:, c, :],
                        rhs=xt_sb[:, c, cols],
                        start=(c == 0),
                        stop=(c == KC - 1),
                    )
                nc.vector.tensor_copy(hT_sb[:, cols], hT_ps)

            # --- per 128-chunk: transpose, z=exp, G=[h*z | z], hi/lo bf16 ---
            ghl_sb = singles.tile([P, RC, 2, EXT], BF16)
            for q in range(RC):
                h_ps = smallpsum.tile([P, EXT], FP32, tag="smallps")
                nc.tensor.transpose(
                    h_ps, hT_sb[:, q * P : (q + 1) * P], ident[:EXT, :EXT]
                )
                g_sb = work.tile([P, EXT], FP32, tag="g")
                z_sb = work.tile([P, H], FP32, tag="z")
                nc.scalar.activation(z_sb, h_ps[:, HF:EXT], AFT.Exp)
                nc.vector.tensor_mul(
                    g_sb[:, 0:HF].rearrange("p (h f) -> p h f", h=H),
                    h_ps[:, 0:HF].rearrange("p (h f) -> p h f", h=H),
                    _bcast_head(z_sb),
                )
                nc.vector.tensor_copy(g_sb[:, HF:EXT], z_sb)
                # hi/lo split: exact bf16 representation of fp32 G
                nc.vector.tensor_copy(ghl_sb[:, q, 0, :], g_sb)
                lo_sb = work.tile([P, EXT], FP32, tag="lo")
                nc.vector.tensor_copy(lo_sb, ghl_sb[:, q, 0, :])
                nc.vector.tensor_sub(lo_sb, g_sb, lo_sb)
                nc.vector.tensor_copy(ghl_sb[:, q, 1, :], lo_sb)
                if q == HB - 1:
                    nc.sync.dma_start(out=g_loc_a[:], in_=ghl_sb[:, :HB])
                    nc.gpsimd.collective_compute(
                        "AllGather",
                        ALU.bypass,
                        replica_groups=[list(range(N_CORES))],
                        ins=[g_loc_a[:]],
                        outs=[g_full_a[:]],
                    )
                elif q == RC - 1:
                    nc.sync.dma_start(out=g_loc_b[:], in_=ghl_sb[:, HB:])
                    nc.gpsimd.collective_compute(
                        "AllGather",
                        ALU.bypass,
                        replica_groups=[list(range(N_CORES))],
                        ins=[g_loc_b[:]],
                        outs=[g_full_b[:]],
                    )

            # --- adjT load (bf16, consumed directly by the PE) ---
            at_sb = singles.tile([P, NC, R], BF16)
            at_view = at[:].rearrange("p (n r) -> p n r", n=NC)
            N_SPLITS = 4
            for s in range(N_SPLITS):
                lo, hi = NC // N_SPLITS * s, NC // N_SPLITS * (s + 1)
                nc.sync.dma_start(out=at_sb[:, lo:hi, :], in_=at_view[:, lo:hi, :])

            # --- load gathered G halves, aggregate: outT += G_n.T @ adjT_n ---
            # g_all_X[p, c, q2, s, e] = (hi,lo)[s] of G[c*R + (q2+off)*128 + p, e]
            g_all_a = singles.tile([P, N_CORES, HB, 2, EXT], BF16)
    
            g_all_b = singles.tile([P, N_CORES, HB, 2, EXT], BF16)
            gfa_view = g_full_a[:].rearrange("c p (q s e) -> p c q s e", q=HB, s=2)
            gfb_view = g_full_b[:].rearrange("c p (q s e) -> p c q s e", q=HB, s=2)
            for s in range(2):
                lo, hi = N_CORES // 2 * s, N_CORES // 2 * (s + 1)
                nc.sync.dma_start(out=g_all_a[:, lo:hi], in_=gfa_view[:, lo:hi])
            for s in range(2):
                lo, hi = N_CORES // 2 * s, N_CORES // 2 * (s + 1)
                nc.sync.dma_start(out=g_all_b[:, lo:hi], in_=gfb_view[:, lo:hi])
            outT_ps = bigpsum.tile([EXT, R], FP32, tag="bigps")
            first = True
            for half, g_all_h, qoff in ((0, g_all_a, 0), (1, g_all_b, HB)):
                for c in range(N_CORES):
                    for q2 in range(HB):
                        n = c * RC + qoff + q2
                        for s in range(2):
                            nc.tensor.matmul(
                                outT_ps,
                                lhsT=g_all_h[:, c, q2, s, :],
                                rhs=at_sb[:, n, :],
                                start=first,
                                stop=(half == 1 and c == N_CORES - 1
                                      and q2 == HB - 1 and s == 1),
                            )
                            first = False
            outT_sb = singles.tile([EXT, R], FP32)
            nc.vector.tensor_copy(outT_sb, outT_ps)

            # --- postprocess, batched per stage across the RC chunks ---
            o_ps = [None] * RC
            for q in range(RC):
                o_ps[q] = smallpsum.tile([P, EXT], FP32, tag="smallps", name=f"o_ps{q}")
                nc.tensor.transpose(
                    o_ps[q], outT_sb[:, q * P : (q + 1) * P], ident[:EXT, :EXT]
                )
            xo = [None] * RC
            for q in range(RC):
                rd = work.tile([P, H], FP32, tag="rd")
                nc.vector.reciprocal(rd, o_ps[q][:, HF:EXT])
                xo[q] = post.tile([P, HF], FP32, tag="xo", name=f"xo{q}")
                nc.vector.tensor_mul(
                    xo[q].rearrange("p (h f) -> p h f", h=H),
                    o_ps[q][:, 0:HF].rearrange("p (h f) -> p h f", h=H),
                    _bcast_head(rd),
                )
            # y = relu(xo) + exp(min(xo, 0))  (= elu + 1; log_softmax shift-safe)
            yo = [None] * RC
            eo = [None] * RC
            for q in range(RC):
                mo = work.tile([P, HF], FP32, tag="mo")
                nc.vector.tensor_scalar_min(mo, xo[q], 0.0)
                eo[q] = post.tile([P, HF], FP32, tag="eo", name=f"eo{q}")
                nc.scalar.activation(eo[q], mo, AFT.Exp)
            for q in range(RC):
                yo[q] = post.tile([P, HF], FP32, tag="yo", name=f"yo{q}")
                nc.vector.scalar_tensor_tensor(
                    out=yo[q], in0=xo[q], scalar=0.0, in1=eo[q],
                    op0=ALU.max, op1=ALU.add,
                )
            # log-softmax over the 64 features (no max subtraction needed:
            # y in (0, ~10], exp stays in fp32 range); batch Exp then Ln to
            # avoid ACT table-set thrash.
            ex = [None] * RC
            sm = [None] * RC
            for q in range(RC):
                ex[q] = post.tile([P, HF], FP32, tag="ex", name=f"ex{q}")
                nc.scalar.activation(ex[q], yo[q], AFT.Exp)
            for q in range(RC):
                sm[q] = post.tile([P, 1], FP32, tag="sm", name=f"sm{q}")
                nc.vector.reduce_sum(sm[q], ex[q], axis=mybir.AxisListType.X)
            out_sb = singles.tile([P, RC, HF], FP32)
            for q in range(RC):
                ls = work.tile([P, 1], FP32, tag="ls")
                nc.scalar.activation(ls, sm[q], AFT.Ln)
                nc.vector.tensor_scalar_sub(out_sb[:, q, :], yo[q], ls)
            nc.sync.dma_start(out=out[:], in_=out_sb)

    # Force all ACT activations (Exp + Ln) onto the one table set containing
    # both, so only ONE ACT_TABLE_LOAD is emitted (early, hidden under DMA)
    # instead of a ~1.3us reload at every Exp<->Ln switch. Set indices must
    # stay aligned with act_info.json, so empty the other sets rather than
    # filtering the list.
    orig_gat = bacc.get_activation_tables

    def _one_set(arch):
        return {
            k: (v if k == "natural_log_exp_and_others" else set())
            for k, v in orig_gat(arch).items()
        }

    bacc.get_activation_tables = _one_set
    try:
        nc.finalize()
    finally:
        bacc.get_activation_tables = orig_gat
    return nc


def _pmajor(a, chunk):
    """[chunk*P, L] -> [P, chunk*L] partition-major layout."""
    n, L = a.shape[0] // P, a.shape[1]
    return np.ascontiguousarray(
        a.reshape(n, P, L).transpose(1, 0, 2).reshape(P, n * L)
    )


def _host_prep(x, adj, W, a_dst, n_nodes):
    """Build per-core input maps."""
    R = n_nodes // N_CORES
    Wd = np.einsum(
        "khf,hf->kh", W.reshape(K_IN, H, F), a_dst, dtype=np.float32
    ).astype(np.float32)
    w_ext = np.concatenate([W, Wd], axis=1).astype(np.float32)  # [1024, 72]
    wt = _pmajor(w_ext, K_IN // P)
    adj_bf = adj.astype(ml_dtypes.bfloat16)  # exact for 0/1
    in_maps = []
    for c in range(N_CORES):
        rows = slice(c * R, (c + 1) * R)
        in_maps.append(
            {
                "xt": _pmajor(np.ascontiguousarray(x[rows].T.astype(np.float32)), K_IN // P),
                "at": _pmajor(np.ascontiguousarray(adj_bf[rows].T), n_nodes // P),
                "wt": wt,
            }
        )
    return in_maps


_BUILT = {}


def run(x, adj, W, a_dst, trace=False):
    n_nodes = x.shape[0]
    R = n_nodes // N_CORES
    RC = R // P
    if n_nodes not in _BUILT:
        _BUILT[n_nodes] = build_bass(n_nodes)
    nc = _BUILT[n_nodes]
    in_maps = _host_prep(x, adj, W, a_dst, n_nodes)
    res = run_bass_kernel_spmd(
        nc, in_maps, list(range(N_CORES)), trace=trace
    )
    blocks = []
    for c in range(N_CORES):
        o = res.results[c]["out"]  # [P, RC*HF] p-major
        blocks.append(
            o.reshape(P, RC, HF).transpose(1, 0, 2).reshape(R, HF)
        )
    return np.concatenate(blocks, axis=0).astype(np.float32), res


def kernel(x, adj, W, a_src, a_dst):
    x = np.asarray(x, dtype=np.float32)
    adj = np.asarray(adj)
    W = np.asarray(W, dtype=np.float32)
    a_dst = np.asarray(a_dst, dtype=np.float32)
    out, _ = run(x, adj, W, a_dst, trace=False)
    return out



# revision 2
# speedup vs baseline: 2.5520x; 2.5520x over previous
"""GAT-style GNN message passing on 8 TRN2 NeuronCores — no collectives.

Math: with LEAK=1 the leaky-relu is identity, so
  e[i,j,h] = e_src[i,h] + e_dst[j,h]
and softmax over j cancels e_src (and any row max) exactly:
  attn[i,j,h] = adj[i,j]*exp(e_dst[j,h]) / sum_j adj[i,j]*exp(e_dst[j,h])
  out[i,(h,f)] = (adj @ (z*h))[i,(h,f)] / (adj @ z)[i,h],  z = exp(e_dst)
then elu + log_softmax per row. log_softmax is shift invariant, so
elu(x) is computed as relu(x) + exp(min(x,0)) (drops the uniform -1).

Sharding: ROW-shard adj/out only; REPLICATE the h computation. The
previous all-gather design lost ~80us to the collective stack (38.7us
entry barrier + 2x ~14us RDH AllGathers for 74KB payloads). Instead
every core loads the full x (fp8, 4MB) and computes h/z/G for all 4096
nodes locally (~14us extra PE time), then aggregates its own
[512, 4096] adjacency slab. Zero cross-core traffic.

Precision: all matmul inputs are fp8 e4m3. adj entries (0/1) are exact
in fp8. x/W quantization errors average out over the 1024-deep (h) and
~2048-deep (aggregation) contractions; measured end-to-end rel err is
~1e-3 vs the 2e-2 gate. W columns are pre-scaled by 8 (and the fused
a_dst columns by 32) to avoid fp8-subnormal truncation; the scales are
divided back out in the on-chip postprocessing (exp has a scale arg,
the g-multiply uses scalar_tensor_tensor).

Per-core device program (R = N/8 = 512 rows, P = 128):
  inputs:  xt [128, 8*8*512] fp8   xt[p, c*4096+k*512+n] = x[c*512+n, k*128+p]
           wt [128, 8*80]    fp8   wt[p, k*80+e] = w_ext[k*128+p, e] (e<72)
           at [128, 32*512]  fp8   at[p, j*512+r] = adj[core*512+r, j*128+p]
  output:  out_p [128, 4*64] f32   out_p[p, q*64+f] = out[core*512+q*128+p, f]

Pipeline: h-matmul (64 MMs, fp8) -> PE transposes of the 32 [72,128]
hT chunks into PSUM (bf16) -> batched exp/multiply builds G=[h*z | z]
in fp8 -> 32 accumulating aggregation MMs against the adj slab -> small
transposed postprocess (reciprocal, elu, log-softmax) -> one output DMA.
"""

import sys

import numpy as np

if "/opt/trn_rl_repo" not in sys.path:
    sys.path.insert(0, "/opt/trn_rl_repo")

import ml_dtypes  # noqa: E402

import concourse.bass as bass  # noqa: E402
import concourse.tile as tile  # noqa: E402
from concourse import bacc, mybir  # noqa: E402
from concourse.bass_utils import run_bass_kernel_spmd  # noqa: E402
from concourse.masks import make_identity  # noqa: E402

N_CORES = 8
N_NODES = 4096
H = 8
F = 8
HF = H * F  # 64
EXT = HF + H  # 72: [h | e_dst]
EXTP = 80  # padded slot width (fp8 bytes) so DoubleRow strides are %16
K_IN = 1024
P = 128
KC = K_IN // P  # 8 k-chunks
CC = N_NODES // 512  # 8 column chunks for the h matmul
NC = N_NODES // P  # 32 j-chunks for the aggregation
R = N_NODES // N_CORES  # 512 rows per core
RC = R // P  # 4 output chunks per core

S_W = 8.0  # host pre-scale on W columns (fp8 subnormal avoidance)
S_D = 32.0  # host pre-scale on the fused a_dst columns

USE_DOUBLE_ROW = False  # fp8 DoubleRow perf mode on the big matmuls

FP32 = mybir.dt.float32
BF16 = mybir.dt.bfloat16
FP8 = mybir.dt.float8e4
NP_FP8 = ml_dtypes.float8_e4m3
AFT = mybir.ActivationFunctionType
ALU = mybir.AluOpType


def _bcast_f(ap_pch):
    """[..., H] AP -> [..., H, F] AP broadcasting each head value over F."""
    return bass.AP(
        tensor=ap_pch.tensor,
        offset=ap_pch.offset,
        ap=list(ap_pch.ap) + [[0, F]],
    )


def build_bass() -> bass.Bass:
    nc = bacc.Bacc(num_devices=N_CORES)

    xt = nc.declare_dram_parameter("xt", [P, CC * KC * 512], FP8, isOutput=False)
    wt = nc.declare_dram_parameter("wt", [P, KC * EXTP], FP8, isOutput=False)
    at = nc.declare_dram_parameter("at", [P, NC * 512], FP8, isOutput=False)
    out = nc.declare_dram_parameter("out", [P, RC * HF], FP32, isOutput=True)

    with tile.TileContext(nc) as tc:
        with (
            tc.tile_pool(name="singles", bufs=1) as singles,
            tc.tile_pool(name="hps", bufs=2, space="PSUM") as hps,
            tc.tile_pool(name="tps", bufs=1, space="PSUM") as tps,
            tc.tile_pool(name="aps", bufs=1, space="PSUM") as aps,
            tc.tile_pool(name="ops", bufs=1, space="PSUM") as ops,
            tc.tile_pool(name="work", bufs=2) as work,
        ):
            ident_bf = singles.tile([P, P], BF16)
            make_identity(nc, ident_bf)
            ident_f = singles.tile([EXT, EXT], FP32)
            make_identity(nc, ident_f)

            # --- loads (p-major, one contiguous run per partition) ---
            wt_sb = singles.tile([P, KC, EXTP], FP8)
            nc.sync.dma_start(
                out=wt_sb, in_=wt[:].rearrange("p (k e) -> p k e", k=KC)
            )
            xt_sb = singles.tile([P, CC, KC, 512], FP8)
            xt_view = xt[:].rearrange("p (c k n) -> p c k n", c=CC, k=KC)
            for c in range(CC):
                nc.sync.dma_start(out=xt_sb[:, c], in_=xt_view[:, c])
            at_sb = singles.tile([P, NC, 512], FP8)
            at_view = at[:].rearrange("p (j r) -> p j r", j=NC)
            for s in range(2):
                lo, hi = NC // 2 * s, NC // 2 * (s + 1)
                nc.sync.dma_start(out=at_sb[:, lo:hi], in_=at_view[:, lo:hi])

            # --- hT = w_ext.T @ x.T : [72, 4096] fp8 matmuls, fp32 PSUM ---
            hT_sb = singles.tile([EXT, CC, 512], BF16)
            for c in range(CC):
                hT_ps = hps.tile([EXT, 512], FP32, tag="hps", name=f"hT{c}")
                if USE_DOUBLE_ROW:
                    for t in range(KC // 2):
                        nc.tensor.matmul(
                            hT_ps,
                            lhsT=wt_sb[:, 2 * t : 2 * t + 2, :EXT],
                            rhs=xt_sb[:, c, 2 * t : 2 * t + 2, :],
                            start=(t == 0),
                            stop=(t == KC // 2 - 1),
                            perf_mode=mybir.MatmulPerfMode.DoubleRow,
                        )
                else:
                    for k in range(KC):
                        nc.tensor.matmul(
                            hT_ps,
                            lhsT=wt_sb[:, k, :EXT],
                            rhs=xt_sb[:, c, k, :],
                            start=(k == 0),
                            stop=(k == KC - 1),
                        )
                # evacuate to bf16, alternating engines for balance
                if c % 2 == 0:
                    nc.vector.tensor_copy(hT_sb[:, c, :], hT_ps)
                else:
                    nc.scalar.activation(hT_sb[:, c, :], hT_ps, AFT.Copy)

            # --- transpose the 32 [72,128] chunks -> [128, 72] (bf16 PSUM) ---
            tr_ps = tps.tile([P, NC, P], BF16)  # 256B slots, 8/bank, 4 banks
            for c in range(CC):
                for q in range(4):
                    j = c * 4 + q
                    nc.tensor.transpose(
                        tr_ps[:, j, :EXT],
                        hT_sb[:, c, q * P : (q + 1) * P],
                        ident_bf[:EXT, :EXT],
                    )

            # --- z = exp(e), G = [h*z | z] in fp8, in two halves ---
            z_all = singles.tile([P, NC, H], BF16)
            g_ext = singles.tile([P, NC, EXTP], FP8)
            for s in range(2):
                sl = slice(NC // 2 * s, NC // 2 * (s + 1))
                nc.scalar.activation(
                    z_all[:, sl, :], tr_ps[:, sl, HF:EXT], AFT.Exp, scale=1.0 / S_D
                )
                nc.vector.scalar_tensor_tensor(
                    out=g_ext[:, sl, 0:HF].rearrange("p c (h f) -> p c h f", h=H),
                    in0=tr_ps[:, sl, 0:HF].rearrange("p c (h f) -> p c h f", h=H),
                    scalar=1.0 / S_W,
                    in1=_bcast_f(z_all[:, sl, :]),
                    op0=ALU.mult,
                    op1=ALU.mult,
                )
                nc.vector.tensor_copy(g_ext[:, sl, HF:EXT], z_all[:, sl, :])

            # --- aggregation: outT[72, 512] += G_j.T @ adjT_j over 32 chunks ---
            outT_ps = aps.tile([EXT, 512], FP32)
            if USE_DOUBLE_ROW:
                for t in range(NC // 2):
                    nc.tensor.matmul(
                        outT_ps,
                        lhsT=g_ext[:, 2 * t : 2 * t + 2, 0:EXT],
                        rhs=at_sb[:, 2 * t : 2 * t + 2, :],
                        start=(t == 0),
                        stop=(t == NC // 2 - 1),
                        perf_mode=mybir.MatmulPerfMode.DoubleRow,
                    )
            else:
                for j in range(NC):
                    nc.tensor.matmul(
                        outT_ps,
                        lhsT=g_ext[:, j, 0:EXT],
                        rhs=at_sb[:, j, :],
                        start=(j == 0),
                        stop=(j == NC - 1),
                    )
            outT_sb = singles.tile([EXT, 512], FP32)
            nc.vector.tensor_copy(outT_sb, outT_ps)

            # --- postprocess: x = num/den, elu+1, log_softmax ---
            o_ps = ops.tile([P, RC, P], FP32)  # 512B slots, 4/bank, 1 bank
            for q in range(RC):
                nc.tensor.transpose(
                    o_ps[:, q, :EXT],
                    outT_sb[:, q * P : (q + 1) * P],
                    ident_f,
                )
            rd = work.tile([P, RC, H], FP32, tag="rd")
            nc.vector.reciprocal(rd, o_ps[:, :, HF:EXT])
            xo = work.tile([P, RC, HF], FP32, tag="xo")
            nc.vector.tensor_mul(
                xo[:].rearrange("p q (h f) -> p q h f", h=H),
                o_ps[:, :, 0:HF].rearrange("p q (h f) -> p q h f", h=H),
                _bcast_f(rd[:]),
            )
            # y = relu(x) + exp(min(x, 0))  (= elu + 1; log_softmax shift-safe)
            mo = work.tile([P, RC, HF], FP32, tag="mo")
            nc.vector.tensor_scalar_min(mo, xo, 0.0)
            eo = work.tile([P, RC, HF], FP32, tag="eo")
            nc.scalar.activation(eo, mo, AFT.Exp)
            yo = work.tile([P, RC, HF], FP32, tag="yo")
            nc.vector.scalar_tensor_tensor(
                out=yo, in0=xo, scalar=0.0, in1=eo, op0=ALU.max, op1=ALU.add
            )
            ex = work.tile([P, RC, HF], FP32, tag="ex")
            nc.scalar.activation(ex, yo, AFT.Exp)
            out_sb = singles.tile([P, RC, HF], FP32)
            for q in range(RC):
                sm = work.tile([P, 1], FP32, tag="sm", name=f"sm{q}")
                nc.vector.reduce_sum(sm, ex[:, q, :], axis=mybir.AxisListType.X)
                ls = work.tile([P, 1], FP32, tag="ls", name=f"ls{q}")
                nc.scalar.activation(ls, sm, AFT.Ln)
                nc.vector.tensor_scalar_sub(out_sb[:, q, :], yo[:, q, :], ls)
            nc.sync.dma_start(out=out[:], in_=out_sb)

    # Force all ACT activations (Exp + Ln) onto the one table set containing
    # both, so only ONE ACT_TABLE_LOAD is emitted (early, hidden under DMA)
    # instead of a ~1.3us reload at every Exp<->Ln switch.
    orig_gat = bacc.get_activation_tables

    def _one_set(arch):
        return {
            k: (v if k == "natural_log_exp_and_others" else set())
            for k, v in orig_gat(arch).items()
        }

    bacc.get_activation_tables = _one_set
    try:
        nc.finalize()
    finally:
        bacc.get_activation_tables = orig_gat
    return nc


def _host_prep(x, adj, W, a_dst):
    """Build per-core input maps (xt/wt replicated, at row-sharded)."""
    Wd = np.einsum(
        "khf,hf->kh", W.reshape(K_IN, H, F), a_dst, dtype=np.float32
    ).astype(np.float32)
    w_ext = np.concatenate([W * S_W, Wd * S_D], axis=1)  # [1024, 72]
    wt_np = np.zeros((P, KC, EXTP), dtype=NP_FP8)
    wt_np[:, :, :EXT] = (
        w_ext.reshape(KC, P, EXT).transpose(1, 0, 2).astype(NP_FP8)
    )
    wt_np = wt_np.reshape(P, KC * EXTP)

    x8 = x.astype(NP_FP8)  # [4096, 1024]
    # xt[p, c, k, n] = x8[c*512+n, k*128+p]
    xt_np = np.ascontiguousarray(
        x8.reshape(CC, 512, KC, P).transpose(3, 0, 2, 1)
    ).reshape(P, CC * KC * 512)

    adj8 = (adj > 0).astype(NP_FP8)  # [4096, 4096]
    in_maps = []
    for c in range(N_CORES):
        rows = slice(c * R, (c + 1) * R)
        # at[p, j, r] = adj8[c*R+r, j*128+p]
        at_np = np.ascontiguousarray(
            adj8[rows].reshape(R, NC, P).transpose(2, 1, 0)
        ).reshape(P, NC * R)
        in_maps.append({"xt": xt_np, "wt": wt_np, "at": at_np})
    return in_maps


_BUILT = {}


def run(x, adj, W, a_dst, trace=False):
    if "nc" not in _BUILT:
        _BUILT["nc"] = build_bass()
    nc = _BUILT["nc"]
    in_maps = _host_prep(x, adj, W, a_dst)
    res = run_bass_kernel_spmd(nc, in_maps, list(range(N_CORES)), trace=trace)
    blocks = []
    for c in range(N_CORES):
        o = res.results[c]["out"]  # [P, RC*HF] p-major
        blocks.append(o.reshape(P, RC, HF).transpose(1, 0, 2).reshape(R, HF))
    return np.concatenate(blocks, axis=0).astype(np.float32), res


def kernel(x, adj, W, a_src, a_dst):
    x = np.asarray(x, dtype=np.float32)
    adj = np.asarray(adj)
    W = np.asarray(W, dtype=np.float32)
    a_dst = np.asarray(a_dst, dtype=np.float32)
    out, _ = run(x, adj, W, a_dst, trace=False)
    return out


# revision 6
# speedup vs baseline: 3.1629x; 1.2394x over previous
"""GAT-style GNN message passing on 8 TRN2 NeuronCores — no collectives.

Math: with LEAK=1 the leaky-relu is identity, so
  e[i,j,h] = e_src[i,h] + e_dst[j,h]
and softmax over j cancels e_src (and any row max) exactly:
  attn[i,j,h] = adj[i,j]*exp(e_dst[j,h]) / sum_j adj[i,j]*exp(e_dst[j,h])
  out[i,(h,f)] = (adj @ (z*h))[i,(h,f)] / (adj @ z)[i,h],  z = exp(e_dst)
then elu + log_softmax per row. log_softmax is shift invariant, so
elu(x) is computed as relu(x) + exp(min(x,0)) (drops the uniform -1).

Sharding: ROW-shard adj/out only; REPLICATE the h computation. The
previous all-gather design lost ~80us to the collective stack (38.7us
entry barrier + 2x ~14us RDH AllGathers for 74KB payloads). Instead
every core loads the full x (fp8, 4MB) and computes h/z/G for all 4096
nodes locally (~14us extra PE time), then aggregates its own
[512, 4096] adjacency slab. Zero cross-core traffic.

Precision: all matmul inputs are fp8 e4m3. adj entries (0/1) are exact
in fp8. x/W quantization errors average out over the 1024-deep (h) and
~2048-deep (aggregation) contractions; measured end-to-end rel err is
~1e-3 vs the 2e-2 gate. W columns are pre-scaled by 8 (and the fused
a_dst columns by 32) to avoid fp8-subnormal truncation; the scales are
divided back out in the on-chip postprocessing (exp has a scale arg,
the g-multiply uses scalar_tensor_tensor).

Per-core device program (R = N/8 = 512 rows, P = 128):
  inputs:  xt [128, 8*8*512] fp8   xt[p, c*4096+k*512+n] = x[c*512+n, k*128+p]
           wt [128, 8*80]    fp8   wt[p, k*80+e] = w_ext[k*128+p, e] (e<72)
           at [128, 32*512]  fp8   at[p, j*512+r] = adj[core*512+r, j*128+p]
  output:  out_p [128, 4*64] f32   out_p[p, q*64+f] = out[core*512+q*128+p, f]

Pipeline: h-matmul (64 MMs, fp8) -> PE transposes of the 32 [72,128]
hT chunks into PSUM (bf16) -> batched exp/multiply builds G=[h*z | z]
in fp8 -> 32 accumulating aggregation MMs against the adj slab -> small
transposed postprocess (reciprocal, elu, log-softmax) -> one output DMA.
"""

import sys

import numpy as np

if "/opt/trn_rl_repo" not in sys.path:
    sys.path.insert(0, "/opt/trn_rl_repo")

import ml_dtypes  # noqa: E402

import concourse.bass as bass  # noqa: E402
import concourse.tile as tile  # noqa: E402
from concourse import bacc, mybir  # noqa: E402
from concourse.bass_utils import run_bass_kernel_spmd  # noqa: E402
from concourse.masks import make_identity  # noqa: E402

N_CORES = 8
N_NODES = 4096
H = 8
F = 8
HF = H * F  # 64
EXT = HF + H  # 72: [h | e_dst]
EXTP = 80  # padded slot width (fp8 bytes) so DoubleRow strides are %16
K_IN = 1024
P = 128
KC = K_IN // P  # 8 k-chunks
CC = N_NODES // 512  # 8 column chunks for the h matmul
NC = N_NODES // P  # 32 j-chunks for the aggregation
R = N_NODES // N_CORES  # 512 rows per core
RC = R // P  # 4 output chunks per core

S_W = 8.0  # host pre-scale on W columns (fp8 subnormal avoidance)
S_D = 32.0  # host pre-scale on the fused a_dst columns

USE_DOUBLE_ROW = True  # fp8 DoubleRow perf mode on the big matmuls
N_WARMUP_MM = 26  # dummy matmuls to trip the PE HAM clock gate early

FP32 = mybir.dt.float32
BF16 = mybir.dt.bfloat16
FP8 = mybir.dt.float8e4
NP_FP8 = ml_dtypes.float8_e4m3
AFT = mybir.ActivationFunctionType
ALU = mybir.AluOpType


def _bcast_f(ap_pch):
    """[..., H] AP -> [..., H, F] AP broadcasting each head value over F."""
    return bass.AP(
        tensor=ap_pch.tensor,
        offset=ap_pch.offset,
        ap=list(ap_pch.ap) + [[0, F]],
    )


def build_bass() -> bass.Bass:
    nc = bacc.Bacc(num_devices=N_CORES)

    xt = nc.declare_dram_parameter("xt", [P, CC * KC * 512], FP8, isOutput=False)
    wt = nc.declare_dram_parameter("wt", [P, KC * EXTP], FP8, isOutput=False)
    at = nc.declare_dram_parameter("at", [P, NC * 512], FP8, isOutput=False)
    out = nc.declare_dram_parameter("out", [P, RC * HF], FP32, isOutput=True)

    with tile.TileContext(nc) as tc:
        with (
            tc.tile_pool(name="singles", bufs=1) as singles,
            tc.tile_pool(name="hps", bufs=2, space="PSUM") as hps,
            tc.tile_pool(name="tps", bufs=1, space="PSUM") as tps,
            tc.tile_pool(name="aps", bufs=1, space="PSUM") as aps,
            tc.tile_pool(name="ops", bufs=1, space="PSUM") as ops,
            tc.tile_pool(name="work", bufs=2) as work,
        ):
            ident_bf = singles.tile([P, P], BF16)
            make_identity(nc, ident_bf)
            ident_f = singles.tile([EXT, EXT], FP32)
            make_identity(nc, ident_f)

            # --- loads (p-major, one contiguous run per partition) ---
            # wt goes on the Scalar HWDGE queue so its issue doesn't delay
            # the xt chunk issues on the Sync queue (each DMA trigger holds
            # its issuing engine ~0.65us).
            wt_sb = singles.tile([P, KC, EXTP], FP8)
            nc.scalar.dma_start(
                out=wt_sb, in_=wt[:].rearrange("p (k e) -> p k e", k=KC)
            )
            xt_sb = singles.tile([P, CC, KC, 512], FP8)
            xt_view = xt[:].rearrange("p (c k n) -> p c k n", c=CC, k=KC)
            for c in range(0, CC, 2):  # 1MB chunks: past the DMA-size knee
                nc.sync.dma_start(out=xt_sb[:, c : c + 2], in_=xt_view[:, c : c + 2])
            at_sb = singles.tile([P, NC, 512], FP8)
            at_view = at[:].rearrange("p (j r) -> p j r", j=NC)
            for s in range(2):
                lo, hi = NC // 2 * s, NC // 2 * (s + 1)
                nc.sync.dma_start(out=at_sb[:, lo:hi], in_=at_view[:, lo:hi])

            # --- postprocess PSUM tile, also used as warmup scratch ---
            o_ps = ops.tile([P, RC, P], FP32)  # 512B slots, 4/bank, 1 bank

            # --- PE warmup: real (non-transpose) matmuls on the identity to
            # trip the HAM activity window while the first xt DMA is in
            # flight, so the real matmuls run at 2.4GHz from the start ---
            for i in range(N_WARMUP_MM):
                nc.tensor.matmul(
                    o_ps[0:64, 0, :],
                    lhsT=ident_bf[:, 0:64],
                    rhs=ident_bf[:, :],
                    start=True,
                    stop=True,
                )

            # --- hT = w_ext.T @ x.T : [72, 4096] fp8 matmuls, fp32 PSUM,
            # with the PE transposes of chunk c-1 interleaved after the
            # matmuls of chunk c to keep the PE stream dense ---
            hT_sb = singles.tile([EXT, CC, 512], BF16)
            tr_ps = tps.tile([P, NC, P], BF16)  # 256B slots, 8/bank, 4 banks

            def do_transposes(c):
                for q in range(4):
                    j = c * 4 + q
                    nc.tensor.transpose(
                        tr_ps[:, j, :EXT],
                        hT_sb[:, c, q * P : (q + 1) * P],
                        ident_bf[:EXT, :EXT],
                    )

            for c in range(CC):
                hT_ps = hps.tile([EXT, 512], FP32, tag="hps", name=f"hT{c}")
                if USE_DOUBLE_ROW:
                    for t in range(KC // 2):
                        nc.tensor.matmul(
                            hT_ps,
                            lhsT=wt_sb[:, 2 * t : 2 * t + 2, :EXT],
                            rhs=xt_sb[:, c, 2 * t : 2 * t + 2, :],
                            start=(t == 0),
                            stop=(t == KC // 2 - 1),
                            perf_mode=mybir.MatmulPerfMode.DoubleRow,
                        )
                else:
                    for k in range(KC):
                        nc.tensor.matmul(
                            hT_ps,
                            lhsT=wt_sb[:, k, :EXT],
                            rhs=xt_sb[:, c, k, :],
                            start=(k == 0),
                            stop=(k == KC - 1),
                        )
                # evacuate to bf16, alternating engines for balance
                if c % 2 == 0:
                    nc.vector.tensor_copy(hT_sb[:, c, :], hT_ps)
                else:
                    nc.scalar.activation(hT_sb[:, c, :], hT_ps, AFT.Copy)
                if c >= 1:
                    do_transposes(c - 1)
            do_transposes(CC - 1)

            # --- z = exp(e), G = [h*z | z] in fp8, in two halves ---
            z_all = singles.tile([P, NC, H], BF16)
            g_ext = singles.tile([P, NC, EXTP], FP8)
            for s in range(2):
                sl = slice(NC // 2 * s, NC // 2 * (s + 1))
                nc.scalar.activation(
                    z_all[:, sl, :], tr_ps[:, sl, HF:EXT], AFT.Exp, scale=1.0 / S_D
                )
                nc.vector.scalar_tensor_tensor(
                    out=g_ext[:, sl, 0:HF].rearrange("p c (h f) -> p c h f", h=H),
                    in0=tr_ps[:, sl, 0:HF].rearrange("p c (h f) -> p c h f", h=H),
                    scalar=1.0 / S_W,
                    in1=_bcast_f(z_all[:, sl, :]),
                    op0=ALU.mult,
                    op1=ALU.mult,
                )
                nc.vector.tensor_copy(g_ext[:, sl, HF:EXT], z_all[:, sl, :])

            # --- aggregation: outT[72, 512] += G_j.T @ adjT_j over 32 chunks ---
            outT_ps = aps.tile([EXT, 512], FP32)
            if USE_DOUBLE_ROW:
                for t in range(NC // 2):
                    nc.tensor.matmul(
                        outT_ps,
                        lhsT=g_ext[:, 2 * t : 2 * t + 2, 0:EXT],
                        rhs=at_sb[:, 2 * t : 2 * t + 2, :],
                        start=(t == 0),
                        stop=(t == NC // 2 - 1),
                        perf_mode=mybir.MatmulPerfMode.DoubleRow,
                    )
            else:
                for j in range(NC):
                    nc.tensor.matmul(
                        outT_ps,
                        lhsT=g_ext[:, j, 0:EXT],
                        rhs=at_sb[:, j, :],
                        start=(j == 0),
                        stop=(j == NC - 1),
                    )
            outT_sb = singles.tile([EXT, 512], FP32)
            nc.vector.tensor_copy(outT_sb, outT_ps)

            # --- postprocess: x = num/den, elu+1, log_softmax ---
            for q in range(RC):
                nc.tensor.transpose(
                    o_ps[:, q, :EXT],
                    outT_sb[:, q * P : (q + 1) * P],
                    ident_f,
                )
            rd = work.tile([P, RC, H], FP32, tag="rd")
            nc.vector.reciprocal(rd, o_ps[:, :, HF:EXT])
            xo = work.tile([P, RC, HF], FP32, tag="xo")
            nc.vector.tensor_mul(
                xo[:].rearrange("p q (h f) -> p q h f", h=H),
                o_ps[:, :, 0:HF].rearrange("p q (h f) -> p q h f", h=H),
                _bcast_f(rd[:]),
            )
            # y = relu(x) + exp(min(x, 0))  (= elu + 1; log_softmax shift-safe)
            mo = work.tile([P, RC, HF], FP32, tag="mo")
            nc.vector.tensor_scalar_min(mo, xo, 0.0)
            eo = work.tile([P, RC, HF], FP32, tag="eo")
            nc.scalar.activation(eo, mo, AFT.Exp)
            yo = work.tile([P, RC, HF], FP32, tag="yo")
            nc.vector.scalar_tensor_tensor(
                out=yo, in0=xo, scalar=0.0, in1=eo, op0=ALU.max, op1=ALU.add
            )
            ex = work.tile([P, RC, HF], FP32, tag="ex")
            nc.scalar.activation(ex, yo, AFT.Exp)
            sm = work.tile([P, RC], FP32, tag="sm")
            nc.vector.reduce_sum(sm, ex, axis=mybir.AxisListType.X)
            ls = work.tile([P, RC], FP32, tag="ls")
            nc.scalar.activation(ls, sm, AFT.Ln)
            out_sb = singles.tile([P, RC, HF], FP32)
            ls_b = bass.AP(
                tensor=ls[:].tensor,
                offset=ls[:].offset,
                ap=list(ls[:].ap) + [[0, HF]],
            )
            nc.vector.tensor_sub(out_sb, yo, ls_b)
            nc.sync.dma_start(out=out[:], in_=out_sb)

    # Force all ACT activations (Exp + Ln) onto the one table set containing
    # both, so only ONE ACT_TABLE_LOAD is emitted (early, hidden under DMA)
    # instead of a ~1.3us reload at every Exp<->Ln switch.
    orig_gat = bacc.get_activation_tables

    def _one_set(arch):
        return {
            k: (v if k == "natural_log_exp_and_others" else set())
            for k, v in orig_gat(arch).items()
        }

    bacc.get_activation_tables = _one_set
    try:
        nc.finalize()
    finally:
        bacc.get_activation_tables = orig_gat
    return nc


def _host_prep(x, adj, W, a_dst):
    """Build per-core input maps (xt/wt replicated, at row-sharded)."""
    Wd = np.einsum(
        "khf,hf->kh", W.reshape(K_IN, H, F), a_dst, dtype=np.float32
    ).astype(np.float32)
    w_ext = np.concatenate([W * S_W, Wd * S_D], axis=1)  # [1024, 72]
    wt_np = np.zeros((P, KC, EXTP), dtype=NP_FP8)
    wt_np[:, :, :EXT] = (
        w_ext.reshape(KC, P, EXT).transpose(1, 0, 2).astype(NP_FP8)
    )
    wt_np = wt_np.reshape(P, KC * EXTP)

    x8 = x.astype(NP_FP8)  # [4096, 1024]
    # xt[p, c, k, n] = x8[c*512+n, k*128+p]
    xt_np = np.ascontiguousarray(
        x8.reshape(CC, 512, KC, P).transpose(3, 0, 2, 1)
    ).reshape(P, CC * KC * 512)

    adj8 = (adj > 0).astype(NP_FP8)  # [4096, 4096]
    in_maps = []
    for c in range(N_CORES):
        rows = slice(c * R, (c + 1) * R)
        # at[p, j, r] = adj8[c*R+r, j*128+p]
        at_np = np.ascontiguousarray(
            adj8[rows].reshape(R, NC, P).transpose(2, 1, 0)
        ).reshape(P, NC * R)
        in_maps.append({"xt": xt_np, "wt": wt_np, "at": at_np})
    return in_maps


_BUILT = {}


def run(x, adj, W, a_dst, trace=False):
    if "nc" not in _BUILT:
        _BUILT["nc"] = build_bass()
    nc = _BUILT["nc"]
    in_maps = _host_prep(x, adj, W, a_dst)
    res = run_bass_kernel_spmd(nc, in_maps, list(range(N_CORES)), trace=trace)
    blocks = []
    for c in range(N_CORES):
        o = res.results[c]["out"]  # [P, RC*HF] p-major
        blocks.append(o.reshape(P, RC, HF).transpose(1, 0, 2).reshape(R, HF))
    return np.concatenate(blocks, axis=0).astype(np.float32), res


def kernel(x, adj, W, a_src, a_dst):
    x = np.asarray(x, dtype=np.float32)
    adj = np.asarray(adj)
    W = np.asarray(W, dtype=np.float32)
    a_dst = np.asarray(a_dst, dtype=np.float32)
    out, _ = run(x, adj, W, a_dst, trace=False)
    return out
